# revision 43
# baseline (speedup 1.0000x reference)
"""Joint attention layer on 8 trn2 NeuronCores (query-sharded, SPMD).

Math (reference):
    Q = img @ Wq.T ; K = text @ Wk.T ; S = Q @ K.T        [N, N]
    attn = softmax(S, axis=1) / sqrt(D)
    out_img = attn @ img ; out_text = attn @ text

Per-core plan (core c owns query rows m in [c*1024, (c+1)*1024)):
    H[j,i]  = sum_d Wq[d,j] Wk[d,i]              (= Wq.T @ Wk, 256x256)
    G[i,m]  = sum_j H[j,i] imgT[j,m]             (absorbs both projections)
    S^T[n,m] = sum_i text[n,i] G[i,m]            (keys on partitions)
    P^T = exp(S^T)  (no max subtraction needed: |S| <~ 55 << 88)
    O[m,:] = sum_n P^T[n,m] * [img|text][n,:]    (PSUM accum over all n)
    rowsum[m] = sum_n P^T[n,m]
    out[m,:] = O[m,:] / rowsum[m] / sqrt(D)

rowsum never touches the PE in the hot loop: the vector engine keeps a
fp32 accumulator rsacc[k_lane, m] += P^T_chunk (the chunk axis is the
loop, so neither ALU axis is the reduction axis), and the residual
128-partition reduce is 4 one-column matmuls against a [128,1] constant
of 16.0 (folding the 1/sqrt(D) into the reciprocal).

Precision: S-chain (Wq,Wk,H,imgT,G,textT) in fp16 (values are O(1));
P^T and the O matmul in bf16 (exp values reach ~e^55, beyond fp16 range);
all accumulation in fp32 (PSUM or vector ALU); epilogue scales in fp32
and stores fp16 (outputs are O(1), halves the output DMA traffic).

The host pre-packs every input so each SBUF tile is filled by exactly
one DMA (83 input DMAs total).  The setup-critical wqk/imgT transfers
ride an otherwise-empty queue and every bulk DMA is gated behind them
(see gated_bulk_dma) so the H->G chain is not starved by fair-shared
DMA bandwidth.  The epilogue alternates vector/scalar engines and
sync/scalar DMA queues so the final m-block's 4 output tiles drain ~2x
faster.  No collectives: outputs are disjoint row slabs concatenated on
the host.
"""

import numpy as np
import ml_dtypes
from contextlib import ExitStack

import concourse.bass as bass
import concourse.tile as tile
from concourse import bacc, mybir
from concourse.bass_utils import run_bass_kernel_spmd

F32 = mybir.dt.float32
F16 = mybir.dt.float16
BF16 = mybir.dt.bfloat16
P = 128          # partitions
D = 256          # hidden dim
N = 8192         # sequence length
N_CORES = 8
SLAB = N // N_CORES          # 1024 query rows per core
MB = 2                       # m-blocks per core
MBS = SLAB // MB             # 512 queries per m-block
NSUB = MBS // P              # 4 psum subtiles per m-block
NCH = N // P                 # 64 key chunks of 128
TTG = 16                     # packed textT tiles
TTW = 2 * N // TTG           # 1024 cols per packed tile (both halves)
TCH = NCH // TTG             # 4 key chunks per packed tile
PIPE = 2                     # S-stage lookahead (chunks)
RTD = 16.0                   # sqrt(D); recip of (RTD * rowsum) = softmax scale

_CACHE = {}


def _build_nc():
    nc = bacc.Bacc("TRN2", target_bir_lowering=False, debug=False,
                   num_devices=N_CORES)

    # it_bf16[n]  = [img[n] | text[n]]                  (one DMA per key chunk)
    # hpk_f16     = [H[0:128] | H[128:256]] col-concat, H = Wq.T @ Wk (host
    #               weight folding -- depends only on the weights)
    # imgT2_f16   = [imgT[0:128] | imgT[128:256]] col-concat (this core's slab)
    # tt2_f16     = groups of [textT[0:128,cols] | textT[128:256,cols]]
    itb_d = nc.dram_tensor("it_bf16", [N, 2 * D], BF16, kind="ExternalInput").ap()
    tt2_d = nc.dram_tensor("tt2_f16", [P, TTG * TTW], F16, kind="ExternalInput").ap()
    hpk_d = nc.dram_tensor("hpk_f16", [P, 2 * D], F16, kind="ExternalInput").ap()
    imgT2_d = nc.dram_tensor("imgT2_f16", [P, 2 * SLAB], F16, kind="ExternalInput").ap()
    out_d = nc.dram_tensor("out", [SLAB, 2 * D], F16, kind="ExternalOutput").ap()

    with tile.TileContext(nc) as tc:
        with ExitStack() as ctx:
            const = ctx.enter_context(tc.tile_pool(name="const", bufs=1))
            rhs_pool = ctx.enter_context(tc.tile_pool(name="rhs", bufs=NCH))

            # ---- all input DMAs up front ----
            # sync queue: only the setup-critical weights -> lowest latency
            # (scalar would delay them behind its ACT_TABLE_LOAD)
            # imgT split per (jt, hh): G's first half (hh=0) only needs two
            # of the four pieces, and smaller transfers complete earlier
            # under the DMA's per-transfer fair sharing
            hpk_sb = const.tile([P, 2 * D], F16, name="hpk")
            imgT_sb = [[const.tile([P, MBS], F16, name=f"imgT{t}_{hh}")
                        for hh in range(2)] for t in range(2)]
            nc.sync.dma_start(hpk_sb[:], hpk_d[:, :])
            for hh in range(2):
                for t in range(2):
                    nc.sync.dma_start(
                        imgT_sb[t][hh][:],
                        imgT2_d[:, t * SLAB + hh * MBS:t * SLAB + (hh + 1) * MBS])

            sixteen_sb = const.tile([P, 1], BF16, name="sixteen")
            nc.vector.memset(sixteen_sb[:], RTD)

            # g split per (it, hh) so mb=0's S matmuls depend only on hh=0
            g_sb = [[const.tile([P, MBS], F16, name=f"g{it}_{hh}")
                     for hh in range(2)] for it in range(2)]

            # ---- setup: G[i,m] = sum_j H[j,i] imgT[j,m]  (H folded on host)
            # 4 psum bufs so the four G pairs run back-to-back instead of
            # serializing through cast+semaphore bank recycling (the pool
            # closes before the main pools open, so the banks are free)
            with tc.tile_pool(name="psetup", bufs=4, space="PSUM") as psetup:
                for hh in range(2):
                    for it in range(2):
                        gp = psetup.tile([P, MBS], F32, tag="g", name=f"gp{it}_{hh}")
                        for jt in range(2):
                            nc.tensor.matmul(
                                gp[:],
                                lhsT=hpk_sb[:, jt * D + it * P:jt * D + (it + 1) * P],
                                rhs=imgT_sb[jt][hh][:],
                                start=(jt == 0), stop=(jt == 1))
                        if it == 0:
                            nc.vector.tensor_copy(g_sb[it][hh][:], gp[:])
                        else:
                            nc.scalar.copy(g_sb[it][hh][:], gp[:])

            # ---- bulk input DMAs (textT groups + img|text row chunks) ----
            # The DMA hardware fair-shares bandwidth across all in-flight
            # transfers, so the setup-critical wqk/imgT DMAs would crawl
            # behind the 12MB bulk.  Gate every bulk DMA on the FIRST
            # critical transfer (wqk) via a one-column prewrite of its
            # destination tile (the DMA then overwrites the whole tile;
            # WAW ordering delays it).  By the time the gated flood's own
            # ~3.5us DMA pipeline latency elapses, all three critical
            # transfers are done.  Prewrites run on the vector engine,
            # which is idle at startup, so the gpsimd queue's DMA issuance
            # is not displaced.  Emitted AFTER the setup compute so the
            # scheduler gives the H/G copies priority over the prewrites.
            def gated_bulk_dma(tile_ap, dram_ap, gated=True):
                if gated:
                    nc.vector.tensor_copy(tile_ap[:, 0:1], hpk_sb[:, 0:1])
                nc.gpsimd.dma_start(tile_ap, dram_ap)

            tt2_sb = [const.tile([P, TTW], F16, name=f"tt2_{g}")
                      for g in range(TTG)]
            rhs_tiles = {}
            for g in range(TTG):
                # the first textT tile and first two row chunks race the
                # criticals ungated: they are needed first and small enough
                # not to starve them
                gated_bulk_dma(tt2_sb[g][:], tt2_d[:, g * TTW:(g + 1) * TTW],
                               gated=(g > 0))
                for ch in range(g * TCH, (g + 1) * TCH):
                    rhs = rhs_pool.tile([P, 2 * D], BF16, tag="rhs",
                                        name=f"rhs{ch}")
                    gated_bulk_dma(rhs[:], itb_d[ch * P:(ch + 1) * P, :],
                                   gated=(ch > 1))
                    rhs_tiles[ch] = rhs

            # ---- main pools ----
            o_pool = ctx.enter_context(tc.tile_pool(name="opool", bufs=5, space="PSUM"))
            s_pool = ctx.enter_context(tc.tile_pool(name="spool", bufs=PIPE + 1, space="PSUM"))
            pt_pool = ctx.enter_context(tc.tile_pool(name="pt", bufs=PIPE + 4))
            eout_pool = ctx.enter_context(tc.tile_pool(name="eout", bufs=4))
            rs_pool = ctx.enter_context(tc.tile_pool(name="rs", bufs=2 * MB))
            rec_pool = ctx.enter_context(tc.tile_pool(name="rec", bufs=MB))

            def s_mm(mb, ch, it, sp):
                g, coff = divmod(ch, TCH)
                coff = it * (TTW // 2) + coff * P
                nc.tensor.matmul(
                    sp[:],
                    lhsT=tt2_sb[g][:, coff:coff + P],
                    rhs=g_sb[it][mb][:],
                    start=(it == 0), stop=(it == 1))

            def s_act(mb, ch, sp):
                pt = pt_pool.tile([P, MBS], BF16, tag="pt", name=f"pt{mb}_{ch}")
                nc.scalar.activation(pt[:], sp[:],
                                     mybir.ActivationFunctionType.Exp)
                return pt

            for mb in range(MB):
                o_ps = [o_pool.tile([P, 2 * D], F32, tag="o", name=f"o{mb}_{i}")
                        for i in range(NSUB)]
                rsacc = rs_pool.tile([P, MBS], F32, tag="rsacc",
                                     name=f"rsacc{mb}")

                pts = {}
                for ch in range(PIPE):
                    sp = s_pool.tile([P, MBS], F32, tag="s", name=f"s{mb}_{ch}")
                    s_mm(mb, ch, 0, sp)
                    s_mm(mb, ch, 1, sp)
                    pts[ch] = s_act(mb, ch, sp)

                for ch in range(NCH):
                    nxt = ch + PIPE
                    sp_n = None
                    if nxt < NCH:
                        sp_n = s_pool.tile([P, MBS], F32, tag="s",
                                           name=f"s{mb}_{nxt}")
                    rhs = rhs_tiles[ch]
                    pt = pts.pop(ch)
                    first, last = (ch == 0), (ch == NCH - 1)

                    def o_mm(sub):
                        nc.tensor.matmul(o_ps[sub][:],
                                         lhsT=pt[:, sub * P:(sub + 1) * P],
                                         rhs=rhs[:], start=first, stop=last)

                    # Interleave fresh-weight MMs (S) between pt-weight O MMs
                    # so every LDWEIGHTS hides under a full 512-col stream.
                    if sp_n is not None:
                        s_mm(mb, nxt, 0, sp_n)
                    o_mm(0)
                    if sp_n is not None:
                        s_mm(mb, nxt, 1, sp_n)
                        pts[nxt] = s_act(mb, nxt, sp_n)
                    o_mm(1)
                    o_mm(2)
                    o_mm(3)

                    # rowsum partials on the vector engine (fp32 accumulate)
                    if first:
                        nc.vector.tensor_copy(rsacc[:], pt[:])
                    else:
                        nc.vector.tensor_tensor(rsacc[:], pt[:], rsacc[:],
                                                op=mybir.AluOpType.add)
                    if last:
                        # residual 128-partition reduce: 4 one-column MMs
                        # against 16.0 -> tr[q, sub] = 16 * rowsum
                        rsb = rs_pool.tile([P, MBS], BF16, tag="rsb",
                                           name=f"rsb{mb}")
                        nc.vector.tensor_copy(rsb[:], rsacc[:])
                        tr_ps = s_pool.tile([P, NSUB], F32, tag="s",
                                            name=f"tr{mb}")
                        for sub in range(NSUB):
                            nc.tensor.matmul(
                                tr_ps[:, sub:sub + 1],
                                lhsT=rsb[:, sub * P:(sub + 1) * P],
                                rhs=sixteen_sb[:],
                                start=(sub == 0), stop=(sub == NSUB - 1),
                                skip_group_check=True)
                        recip = rec_pool.tile([P, NSUB], F32, tag="recip",
                                              name=f"recip{mb}")
                        nc.vector.reciprocal(recip[:], tr_ps[:])

                # epilogue alternates vector/scalar so the 4 tiles drain ~2x
                # faster (only the final m-block's drain is exposed)
                for sub in range(NSUB):
                    osb = eout_pool.tile([P, 2 * D], F16, tag="eout",
                                         name=f"eout{mb}_{sub}")
                    if sub % 2 == 0:
                        nc.vector.tensor_scalar(
                            osb[:], o_ps[sub][:], recip[:, sub:sub + 1], None,
                            op0=mybir.AluOpType.mult)
                    else:
                        nc.scalar.mul(osb[:], o_ps[sub][:],
                                      recip[:, sub:sub + 1])
                    row0 = mb * MBS + sub * P
                    # alternate output queues so the final drain's 4 DMA
                    # issues (~600ns apiece) don't serialize on one engine
                    eng = nc.sync if sub % 2 == 0 else nc.scalar
                    eng.dma_start(out_d[row0:row0 + P, :], osb[:])

    nc.compile()
    return nc


def kernel(img, text, Wq, Wk):
    img = np.ascontiguousarray(img, dtype=np.float32)
    text = np.ascontiguousarray(text, dtype=np.float32)

    if "nc" not in _CACHE:
        _CACHE["nc"] = _build_nc()
    nc = _CACHE["nc"]

    itb = np.concatenate([img.astype(ml_dtypes.bfloat16),
                          text.astype(ml_dtypes.bfloat16)], axis=1)
    textT16 = text.T.astype(np.float16)            # [D, N]
    TW = N // TTG
    tt2 = np.concatenate(
        [np.concatenate([textT16[:P, g * TW:(g + 1) * TW],
                         textT16[P:, g * TW:(g + 1) * TW]],
                        axis=1) for g in range(TTG)], axis=1)
    tt2 = np.ascontiguousarray(tt2)
    # weight folding: H[j,i] = sum_d Wq[d,j] Wk[d,i]
    H = (np.asarray(Wq, dtype=np.float32).T
         @ np.asarray(Wk, dtype=np.float32)).astype(np.float16)
    hpk = np.ascontiguousarray(np.concatenate([H[:P], H[P:]], axis=1))

    in_maps = []
    for c in range(N_CORES):
        imgT = img[c * SLAB:(c + 1) * SLAB].T.astype(np.float16)   # [D, SLAB]
        imgT2 = np.ascontiguousarray(
            np.concatenate([imgT[:P], imgT[P:]], axis=1))
        in_maps.append({
            "it_bf16": itb,
            "tt2_f16": tt2,
            "hpk_f16": hpk,
            "imgT2_f16": imgT2,
        })

    res = run_bass_kernel_spmd(nc, in_maps, core_ids=list(range(N_CORES)),
                               **_CACHE.get("run_kwargs", {}))
    _CACHE["last_results"] = res
    out = np.concatenate([res.results[c]["out"] for c in range(N_CORES)],
                         axis=0).astype(np.float32)
    return np.ascontiguousarray(out[:, :D]), np.ascontiguousarray(out[:, D:])


if __name__ == "__main__":
    rng = np.random.default_rng(0)
    img = rng.standard_normal((N, D), dtype=np.float32)
    text = rng.standard_normal((N, D), dtype=np.float32)
    sc = 1.0 / np.sqrt(D)
    Wq = rng.uniform(-sc, sc, (D, D)).astype(np.float32)
    Wk = rng.uniform(-sc, sc, (D, D)).astype(np.float32)
    oi, ot = kernel(img, text, Wq, Wk)
    print("out_img", oi.shape, oi.dtype, "out_text", ot.shape, ot.dtype)
